# revision 23
# baseline (speedup 1.0000x reference)
"""AR-GAS-Net Trainium2 kernel v3 (8-core SPMD, data-parallel over batch).

Per core (BC=32768 rows):
  - bf16 MLP on TensorE, H padded 100->128 (biases are zero in this net, so
    no bias rows; all stationaries are 128-col -> FWL weight loads).
    x host-cast to bf16 (halves DMA); 4-deep x prefetch keeps PE warm.
  - 64-step GAS scan in G groups; scan(g) overlaps MLP(g+1).
    Chain ops (e,d,r,m1,mu',Q') all on DVE (no cross-engine hops);
    p=e*Q on GpSimd off-chain.  Last group: recip on ACT (bf16 d/r),
    earlier groups: reciprocal_approx_fast on DVE (fp32 d/r).
  - tail: bulk sqrt on ACT; out=dp*sg+mu on DVE; group-0 tail interleaved
    into group-1's scan steps to fill recip-wait gaps.
"""

import os
import numpy as np

import concourse.bass as bass
import concourse.bacc as bacc
import concourse.mybir as mybir
from concourse import tile
from concourse.bass_utils import run_bass_kernel_spmd

f32 = mybir.dt.float32
bf16 = mybir.dt.bfloat16
AF = mybir.ActivationFunctionType
ALU = mybir.AluOpType

B, K, D_IN, H = 262144, 64, 200, 100
HP = 128                    # padded hidden width (zero-bias net)
NCORES = 8
BC = B // NCORES            # 32768 rows per core
P = 128
T = BC // P                 # 256 tiles of 128 rows
G = int(os.environ.get("ARGAS_G", "2"))
TG = T // G                 # tiles per group (free dim of scan ops)
CHUNK = 1024                # MLP chunk rows
NCH = BC // CHUNK           # 32 chunks
CPG = NCH // G              # chunks per group
POOL_OPS = set(filter(None, os.environ.get("ARGAS_POOL", "p").split(",")))
SKEW = int(os.environ.get("ARGAS_SKEW", "32"))  # g1 scan lag (fused middle)
XBUFS = int(os.environ.get("ARGAS_XBUFS", "6"))
# L1/L2 moving-operand width; 1024 is rejected by the NEFF compiler (a
# single matmul may not span PSUM banks), keep 512
MM_N = int(os.environ.get("ARGAS_MMN", "512"))

# ---------------------------------------------------------------- custom ops
_CUSTOM = None


def _register_custom_ops():
    global _CUSTOM
    if _CUSTOM is not None:
        return _CUSTOM
    try:
        import concourse.dve_ops as dve_ops
        from concourse.dve_spec import Spec, Src0, Src1, C0, C1, C2, sq, lower
        from concourse.dve_uop import DveOpSpec

        defs = [
            # d = e*e + Q
            ("ARGAS_SQ_ADD", Spec(
                body=sq(Src0) + Src1,
                reference=lambda in0, in1, c0, c1, c2:
                    in0.astype(np.float32) ** 2 + in1)),
            # mu' = (m1*A + o_mu) + mu*b_mu  (Src0=mu so the strided state
            # AP sits in the unrestricted in0 slot; Src1=m1 stays 1-free-dim)
            ("ARGAS_AFF_AFF2", Spec(
                body=(Src1 * C0 + C1) + Src0 * C2,
                reference=lambda in0, in1, c0, c1, c2:
                    (in1.astype(np.float32) * c0 + c1) + in0 * c2)),
            # Q' = ((Q*r)*c0 + c1)*Q + c2   (Src0=Q reused twice)
            ("ARGAS_QP", Spec(
                body=((Src0 * Src1) * C0 + C1) * Src0 + C2,
                reference=lambda in0, in1, c0, c1, c2:
                    ((in0.astype(np.float32) * in1) * c0 + c1) * in0 + c2)),
        ]
        ops = {}
        for name, spec in defs:
            if name not in dve_ops._SUB_OPCODE_FOR_NAME:
                row = dve_ops._CUSTOM_DVE_ROW_BASE + len(dve_ops.OPS)
                assert row < 0x20, "custom-DVE row overflow"
                dve_ops._SUB_OPCODE_FOR_NAME[name] = row
            tmp = {}
            for ver in ("v3", "v4"):
                try:
                    s = DveOpSpec(
                        name=name,
                        opcode=dve_ops.get_dve_sub_opcode(name),
                        uops=lower(spec, ver=ver),
                        rd1_en=True,
                    )
                    tmp[ver] = s.sha(ver)
                except Exception:
                    pass
            op = dve_ops.DveOp(name, spec, subdim=False, uops_sha=tmp)
            if all(o.name != name for o in dve_ops.OPS):
                dve_ops.OPS.append(op)
            dve_ops.CUSTOM_DVE_SPECS[name] = spec
            ops[name] = op
        _CUSTOM = ops
    except Exception as e:  # pragma: no cover
        print(f"[kernel] custom-DVE registration failed ({e}); using fallback")
        _CUSTOM = {}
    return _CUSTOM


def _act_recip(nc, out, in_):
    """ACT-engine Reciprocal (bypasses the bass accuracy guard; validated
    end-to-end against the reference at the 2e-2 gate)."""
    eng = nc.scalar
    ins = [eng.lower_ap(in_)]
    for val in (0.0, 1.0, 0.0):  # bias, scale, alpha
        ins.append(mybir.ImmediateValue(dtype=mybir.dt.float32, value=val))
    return eng.add_instruction(
        mybir.InstActivation(
            name=eng.bass.get_next_instruction_name(),
            func=AF.Reciprocal,
            ins=ins,
            outs=[eng.lower_ap(out)],
        )
    )


def _dve_recip_bf16(nc, out, in_):
    """reciprocal_approx_fast with bf16 APs (the bass fp32 assert is
    over-strict: DVE converts streams to fp32 before the uop chain, so the
    BITWISE_NOT seed still sees fp32 bit layout)."""
    from concourse.dve_ops import (
        RECIP_APPROX_FAST_CONSTS,
        RECIPROCAL_APPROX_FAST,
    )
    c = RECIP_APPROX_FAST_CONSTS
    return nc.vector._custom_dve(
        RECIPROCAL_APPROX_FAST, out=out, in0=in_,
        s0=c["s0"], s1=c["s1"], imm2=c["imm2"])


# ---------------------------------------------------------------- builder
def build_nc(sc):
    cust = _register_custom_ops()
    assert len(cust) == 3, "custom DVE ops required for this kernel"
    nc = bacc.Bacc(None)

    xT = nc.dram_tensor("xT", [D_IN, BC], bf16, kind="ExternalInput")
    W1d = nc.dram_tensor("W1e", [D_IN, HP], bf16, kind="ExternalInput")
    W2d = nc.dram_tensor("W2e", [HP, HP], bf16, kind="ExternalInput")
    W3d = nc.dram_tensor("W3e", [HP, K], bf16, kind="ExternalInput")
    mu0d = nc.dram_tensor("mu0", [P, T], bf16, kind="ExternalInput")
    s20d = nc.dram_tensor("s20", [P, T], bf16, kind="ExternalInput")
    # k-major bf16 output: out[g, p, k*TG+t] = result[(g*TG+t)*P+p, k];
    # the host transposes/upcasts (keeps every DMA fully contiguous).
    outd = nc.dram_tensor("out", [G, P, K * TG], bf16, kind="ExternalOutput")

    A_ = sc["ns"] * sc["a_mu"] * (1.0 + 1.0 / sc["nu"])
    C_ = sc["ns"] * sc["a_s"] * (1.0 + 1.0 / sc["nu"])
    D_ = sc["b_s"] - sc["ns"] * sc["a_s"]
    Ct = sc["nu"] * C_
    wt = sc["nu"] * sc["o_s"]

    KB = K
    XR = D_IN - P  # 72 rows in the second x slab
    with tile.TileContext(nc) as tc:
        with (
            tc.tile_pool(name="const", bufs=1) as constp,
            tc.tile_pool(name="big", bufs=1) as bigp,
            tc.tile_pool(name="mlp", bufs=XBUFS) as mlpp,
            tc.tile_pool(name="act", bufs=2) as actp,
            tc.tile_pool(name="scan", bufs=3) as scanp,
            tc.tile_pool(name="psmm", bufs=3, space="PSUM") as psmm,
            tc.tile_pool(name="psdp", bufs=2, space="PSUM") as psdp,
        ):
            # ---- constants
            w1a = constp.tile([P, HP], bf16, tag="w1a")
            nc.sync.dma_start(w1a[:], W1d[0:P, :])
            w1b = constp.tile([XR, HP], bf16, tag="w1b")
            nc.sync.dma_start(w1b[:], W1d[P:D_IN, :])
            w2 = constp.tile([HP, HP], bf16, tag="w2")
            nc.sync.dma_start(w2[:], W2d[:])
            w3 = constp.tile([HP, K], bf16, tag="w3")
            nc.sync.dma_start(w3[:], W3d[:])
            zt = constp.tile([P, 1], f32, tag="zt")
            nc.vector.memset(zt[:], 0.0)

            # ---- persistent combined state tiles, k-major within group:
            # DP col g*KB*TG + k*TG + t; MU/QQ col g*(KB+1)*TG + k*TG + t.
            # One tile per state so a single step-sliced AP can address both
            # groups at a constant slab offset (the skew-fused scan).
            DP = bigp.tile([P, G * KB * TG], bf16, tag="DP", name="DP")
            MU = bigp.tile([P, G * (KB + 1) * TG], bf16, tag="MU", name="MU")
            QQ = bigp.tile([P, G * (KB + 1) * TG], bf16, tag="QQ", name="QQ")

            def dp_slab(g, k, n=1):
                b = g * KB * TG
                return DP[:, b + k * TG:b + (k + n) * TG]

            def mu_slab(g, k, n=1):
                b = g * (KB + 1) * TG
                return MU[:, b + k * TG:b + (k + n) * TG]

            def q_slab(g, k, n=1):
                b = g * (KB + 1) * TG
                return QQ[:, b + k * TG:b + (k + n) * TG]

            def mlp_chunk(g, c, evac_l2_dve=False):
                c_glob = g * CPG + c
                col0 = c_glob * CHUNK
                xa = mlpp.tile([P, CHUNK], bf16, tag="xa")
                nc.sync.dma_start(xa[:], xT[0:P, col0:col0 + CHUNK])
                xb = mlpp.tile([XR, CHUNK], bf16, tag="xb")
                nc.sync.dma_start(xb[:], xT[P:D_IN, col0:col0 + CHUNK])

                r1 = actp.tile([HP, CHUNK], bf16, tag="r1")
                r2 = actp.tile([HP, CHUNK], bf16, tag="r2")

                ps1 = psmm.tile([HP, CHUNK], f32, tag="mm")
                for j in range(CHUNK // MM_N):
                    s = slice(j * MM_N, (j + 1) * MM_N)
                    nc.tensor.matmul(ps1[:, s], w1a[:], xa[:, s],
                                     start=True, stop=False)
                for j in range(CHUNK // MM_N):
                    s = slice(j * MM_N, (j + 1) * MM_N)
                    nc.tensor.matmul(ps1[:, s], w1b[:], xb[:, s],
                                     start=False, stop=True)
                nc.scalar.activation(r1[:], ps1[:], AF.Relu,
                                     bias=zt[:, 0:1])

                ps2 = psmm.tile([HP, CHUNK], f32, tag="mm")
                for j in range(CHUNK // MM_N):
                    s = slice(j * MM_N, (j + 1) * MM_N)
                    nc.tensor.matmul(ps2[:, s], w2[:], r1[:, s],
                                     start=True, stop=True)
                if evac_l2_dve:
                    nc.vector.tensor_scalar_max(r2[:], ps2[:], 0.0)
                else:
                    nc.scalar.activation(r2[:], ps2[:], AF.Relu,
                                         bias=zt[:, 0:1])

                psd = psdp.tile([P, (CHUNK // P) * KB], f32, tag="dp")
                for j in range(CHUNK // P):
                    nc.tensor.matmul(psd[:, j * KB:(j + 1) * KB],
                                     r2[:, j * P:(j + 1) * P], w3[:],
                                     start=True, stop=True)
                # evac to k-major dp: dp[:, k*TG + c*8+j] = psd[:, j*KB+k]
                src = psd[:].rearrange("p (j k) -> p k j", k=KB)
                dst = dp_slab(g, 0, KB).rearrange("p (k t) -> p k t", t=TG)
                dst = dst[:, :, c * (CHUNK // P):(c + 1) * (CHUNK // P)]
                nc.scalar.copy(dst, src)

            def scan_ops(yv, mu_p, mu_n, Q_p, Q_n, width):
                """One GAS step on the given APs (solo or fused width)."""
                def tt(name, out, a, b, op):
                    eng = nc.gpsimd if name in POOL_OPS else nc.vector
                    eng.tensor_tensor(out, a, b, op)

                def scr(tag):
                    t = scanp.tile([P, width * TG], bf16, tag=f"{tag}{width}")
                    flat = t[:]
                    if width == 1:
                        return flat, flat
                    return flat.rearrange("p (x t) -> p x t", t=TG), flat

                e, _ = scr("e")
                tt("e", e, yv, mu_p, ALU.subtract)
                d, _ = scr("d")
                nc.vector._custom_dve(cust["ARGAS_SQ_ADD"],
                                      out=d, in0=e, in1=Q_p)
                p, _ = scr("p")
                tt("p", p, e, Q_p, ALU.mult)
                r, r_flat = scr("r")
                _dve_recip_bf16(nc, r, d)
                m1, m1_flat = scr("m1")
                tt("m", m1, p, r, ALU.mult)
                nc.vector._custom_dve(cust["ARGAS_AFF_AFF2"],
                                      out=mu_n, in0=mu_p, in1=m1_flat,
                                      s0=A_, s1=sc["o_mu"], imm2=sc["b_mu"])
                nc.vector._custom_dve(cust["ARGAS_QP"],
                                      out=Q_n, in0=Q_p, in1=r_flat,
                                      s0=-Ct, s1=Ct + D_, imm2=wt)

            def scan_solo(g, k):
                scan_ops(dp_slab(g, k), mu_slab(g, k), mu_slab(g, k + 1),
                         q_slab(g, k), q_slab(g, k + 1), 1)

            def scan_fused(k):
                """g0 at step k, g1 at step k-SKEW, one AP pair per operand."""
                XD, XM = KB - SKEW, KB + 1 - SKEW
                dpv = DP[:].rearrange("p (x t) -> p x t", t=TG)
                muv = MU[:].rearrange("p (x t) -> p x t", t=TG)
                qv = QQ[:].rearrange("p (x t) -> p x t", t=TG)
                scan_ops(
                    dpv[:, k:k + XD + 1:XD, :],
                    muv[:, k:k + XM + 1:XM, :],
                    muv[:, k + 1:k + 1 + XM + 1:XM, :],
                    qv[:, k:k + XM + 1:XM, :],
                    qv[:, k + 1:k + 1 + XM + 1:XM, :], 2)

            def scan_init(g):
                nc.sync.dma_start(mu_slab(g, 0),
                                  mu0d[:, g * TG:(g + 1) * TG])
                nc.sync.dma_start(q_slab(g, 0),
                                  s20d[:, g * TG:(g + 1) * TG])

            def tail_slab(g, k, add_pool=True):
                """sg=sqrt(Q/nu) on ACT, dp = dp*sg + mu in place (slab k)."""
                sgk = q_slab(g, k + 1)
                nc.scalar.activation(sgk, sgk, AF.Sqrt,
                                     bias=zt[:, 0:1], scale=1.0 / sc["nu"])
                dk = dp_slab(g, k)
                nc.gpsimd.tensor_tensor(dk, dk, sgk, ALU.mult)
                eng = nc.gpsimd if add_pool else nc.vector
                eng.tensor_tensor(dk, dk, mu_slab(g, k + 1), ALU.add)

            def tail_bulk(g, k0, k1):
                sgk = q_slab(g, k0 + 1, k1 - k0)
                nc.scalar.activation(sgk, sgk, AF.Sqrt,
                                     bias=zt[:, 0:1], scale=1.0 / sc["nu"])
                dk = dp_slab(g, k0, k1 - k0)
                nc.vector.tensor_tensor(dk, dk, sgk, ALU.mult)
                nc.vector.tensor_tensor(dk, dk, mu_slab(g, k0 + 1, k1 - k0),
                                        ALU.add)

            def dma_out(g, k0, k1):
                nc.sync.dma_start(outd[g, :, k0 * TG:k1 * TG],
                                  dp_slab(g, k0, k1 - k0))

            # ---------------- emission schedule ----------------
            for g in range(G):
                scan_init(g)
            for c in range(CPG):          # group 0 MLP; L2 relu on idle DVE
                mlp_chunk(0, c, evac_l2_dve=True)

            # phase B: solo g0 steps 0..SKEW-1, mlp-g1 interleaved
            nxt_c = 0
            per = max(1, SKEW // CPG)
            for k in range(SKEW):
                scan_solo(0, k)
                if (k + 1) % per == 0 and nxt_c < CPG:
                    mlp_chunk(1, nxt_c)
                    nxt_c += 1
            while nxt_c < CPG:
                mlp_chunk(1, nxt_c)
                nxt_c += 1
            # phase C: fused (g0 at k, g1 at k-SKEW); g0 tail slabs on
            # Pool/ACT as they become final
            for k in range(SKEW, KB):
                scan_fused(k)
                tail_slab(0, k - SKEW)
            dma_out(0, 0, KB - SKEW)
            # phase D: solo g1 drain + rest of g0 tail; squeeze early g1
            # tail slabs into the remaining Pool/ACT slack
            ng1 = 0
            for k in range(KB - SKEW, KB):
                scan_solo(1, k)
                tail_slab(0, k)
                if k % 2 == 0 and ng1 < KB - SKEW - 2:
                    tail_slab(1, ng1)
                    ng1 += 1
            dma_out(0, KB - SKEW, KB)
            if ng1:
                dma_out(1, 0, ng1)
            # phase E: rest of g1 tail in pipelined k-slabs
            for h in range(2):
                k0 = ng1 + h * (KB - ng1) // 2
                k1 = ng1 + (h + 1) * (KB - ng1) // 2
                tail_bulk(1, k0, k1)
                dma_out(1, k0, k1)
    if not nc.is_finalized():
        nc.finalize()
    return nc


# ---------------------------------------------------------------- tracing
def _maybe_enable_trace():
    if os.environ.get("BASS_TRACE") != "1":
        return
    try:
        import sys, types
        try:
            import antenv.axon_hooks as ah
        except ImportError:
            import antenv
            ah = types.ModuleType("antenv.axon_hooks")
            ah._hook = None
            def _set(h):
                ah._hook = h
            def _get():
                return ah._hook
            ah.set_axon_ntff_profile_hook = _set
            ah.get_axon_ntff_profile_hook = _get
            sys.modules["antenv.axon_hooks"] = ah
            antenv.axon_hooks = ah
        if ah.get_axon_ntff_profile_hook() is not None:
            return
        from trn_agent_boot.trn_boot import _ntff_profile_via_ctypes
        import concourse.bass_utils as bu
        bu.upload_artifacts = lambda tmpdir: tmpdir
        ah.set_axon_ntff_profile_hook(
            _ntff_profile_via_ctypes("/opt/axon/libaxon_pjrt.so"))
        print("[kernel] NTFF profile hook installed")
    except Exception as e:
        print(f"[kernel] trace hook unavailable: {e}")


LAST = None  # last BassKernelResults (dev/tracing)


# ---------------------------------------------------------------- entry
def kernel(**inputs):
    import ml_dtypes
    bfl = ml_dtypes.bfloat16
    _maybe_enable_trace()
    x = np.asarray(inputs["x"], np.float32)
    last_mu = np.asarray(inputs["last_mu"], np.float32)
    last_sigma = np.asarray(inputs["last_sigma"], np.float32)
    sc = dict(
        a_mu=float(inputs["alpha_mu"]), a_s=float(inputs["alpha_sigma"]),
        b_mu=float(inputs["beta_mu"]), b_s=float(inputs["beta_sigma"]),
        o_mu=float(inputs["omega_mu"]), o_s=float(inputs["omega_sigma"]),
        nu=float(inputs["nu"]), ns=float(inputs["norm_strength"]),
    )
    # biases are structurally zero in this net (setup_inputs); the padded
    # no-bias-row layout depends on it.
    for bn in ("b1", "b2", "b3"):
        assert float(np.abs(np.asarray(inputs[bn])).max()) == 0.0, \
            f"{bn} != 0 unsupported by padded kernel"

    def pad(w, rows, cols):
        out = np.zeros((rows, cols), np.float32)
        a = np.asarray(w, np.float32)
        out[:a.shape[0], :a.shape[1]] = a
        return out.astype(bfl)

    W1e = pad(inputs["W1"], D_IN, HP)
    W2e = pad(inputs["W2"], HP, HP)
    W3e = pad(inputs["W3"], HP, K)

    nc = build_nc(sc)
    in_maps = []
    for c in range(NCORES):
        sl = slice(c * BC, (c + 1) * BC)
        in_maps.append({
            "xT": np.ascontiguousarray(x[sl].T).astype(bfl),
            "W1e": W1e, "W2e": W2e, "W3e": W3e,
            "mu0": np.ascontiguousarray(
                last_mu[sl].reshape(T, P).T).astype(bfl),
            "s20": np.ascontiguousarray(
                sc["nu"] * last_sigma[sl].reshape(T, P).T).astype(bfl),
        })
    res = run_bass_kernel_spmd(nc, in_maps, list(range(NCORES)))
    global LAST
    LAST = res
    if res.exec_time_ns is not None:
        print(f"HW exec time: {res.exec_time_ns} ns")
    # out[g, p, k*TG+t] -> full[(g*TG+t)*P+p, k]
    parts = []
    for i in range(NCORES):
        o = np.asarray(res.results[i]["out"]).astype(np.float32)
        o = o.reshape(G, P, K, TG).transpose(0, 3, 1, 2).reshape(BC, K)
        parts.append(o)
    return np.concatenate(parts, 0)


# revision 24
# speedup vs baseline: 1.0088x; 1.0088x over previous
"""AR-GAS-Net Trainium2 kernel v3 (8-core SPMD, data-parallel over batch).

Per core (BC=32768 rows):
  - bf16 MLP on TensorE, H padded 100->128 (biases are zero in this net, so
    no bias rows; all stationaries are 128-col -> FWL weight loads).
    x host-cast to bf16 (halves DMA); 4-deep x prefetch keeps PE warm.
  - 64-step GAS scan in G groups; scan(g) overlaps MLP(g+1).
    Chain ops (e,d,r,m1,mu',Q') all on DVE (no cross-engine hops);
    p=e*Q on GpSimd off-chain.  Last group: recip on ACT (bf16 d/r),
    earlier groups: reciprocal_approx_fast on DVE (fp32 d/r).
  - tail: bulk sqrt on ACT; out=dp*sg+mu on DVE; group-0 tail interleaved
    into group-1's scan steps to fill recip-wait gaps.
"""

import os
import numpy as np

import concourse.bass as bass
import concourse.bacc as bacc
import concourse.mybir as mybir
from concourse import tile
from concourse.bass_utils import run_bass_kernel_spmd

f32 = mybir.dt.float32
bf16 = mybir.dt.bfloat16
AF = mybir.ActivationFunctionType
ALU = mybir.AluOpType

B, K, D_IN, H = 262144, 64, 200, 100
HP = 128                    # padded hidden width (zero-bias net)
NCORES = 8
BC = B // NCORES            # 32768 rows per core
P = 128
T = BC // P                 # 256 tiles of 128 rows
G = int(os.environ.get("ARGAS_G", "2"))
TG = T // G                 # tiles per group (free dim of scan ops)
CHUNK = 1024                # MLP chunk rows
NCH = BC // CHUNK           # 32 chunks
CPG = NCH // G              # chunks per group
POOL_OPS = set(filter(None, os.environ.get("ARGAS_POOL", "p").split(",")))
SKEW = int(os.environ.get("ARGAS_SKEW", "32"))  # g1 scan lag (fused middle)
XBUFS = int(os.environ.get("ARGAS_XBUFS", "6"))
# L1/L2 moving-operand width; 1024 is rejected by the NEFF compiler (a
# single matmul may not span PSUM banks), keep 512
MM_N = int(os.environ.get("ARGAS_MMN", "512"))

# ---------------------------------------------------------------- custom ops
_CUSTOM = None


def _register_custom_ops():
    global _CUSTOM
    if _CUSTOM is not None:
        return _CUSTOM
    try:
        import concourse.dve_ops as dve_ops
        from concourse.dve_spec import Spec, Src0, Src1, C0, C1, C2, sq, lower
        from concourse.dve_uop import DveOpSpec

        defs = [
            # d = e*e + Q
            ("ARGAS_SQ_ADD", Spec(
                body=sq(Src0) + Src1,
                reference=lambda in0, in1, c0, c1, c2:
                    in0.astype(np.float32) ** 2 + in1)),
            # mu' = (m1*A + o_mu) + mu*b_mu  (Src0=mu so the strided state
            # AP sits in the unrestricted in0 slot; Src1=m1 stays 1-free-dim)
            ("ARGAS_AFF_AFF2", Spec(
                body=(Src1 * C0 + C1) + Src0 * C2,
                reference=lambda in0, in1, c0, c1, c2:
                    (in1.astype(np.float32) * c0 + c1) + in0 * c2)),
            # Q' = ((Q*r)*c0 + c1)*Q + c2   (Src0=Q reused twice)
            ("ARGAS_QP", Spec(
                body=((Src0 * Src1) * C0 + C1) * Src0 + C2,
                reference=lambda in0, in1, c0, c1, c2:
                    ((in0.astype(np.float32) * in1) * c0 + c1) * in0 + c2)),
        ]
        ops = {}
        for name, spec in defs:
            if name not in dve_ops._SUB_OPCODE_FOR_NAME:
                row = dve_ops._CUSTOM_DVE_ROW_BASE + len(dve_ops.OPS)
                assert row < 0x20, "custom-DVE row overflow"
                dve_ops._SUB_OPCODE_FOR_NAME[name] = row
            tmp = {}
            for ver in ("v3", "v4"):
                try:
                    s = DveOpSpec(
                        name=name,
                        opcode=dve_ops.get_dve_sub_opcode(name),
                        uops=lower(spec, ver=ver),
                        rd1_en=True,
                    )
                    tmp[ver] = s.sha(ver)
                except Exception:
                    pass
            op = dve_ops.DveOp(name, spec, subdim=False, uops_sha=tmp)
            if all(o.name != name for o in dve_ops.OPS):
                dve_ops.OPS.append(op)
            dve_ops.CUSTOM_DVE_SPECS[name] = spec
            ops[name] = op
        _CUSTOM = ops
    except Exception as e:  # pragma: no cover
        print(f"[kernel] custom-DVE registration failed ({e}); using fallback")
        _CUSTOM = {}
    return _CUSTOM


def _act_recip(nc, out, in_):
    """ACT-engine Reciprocal (bypasses the bass accuracy guard; validated
    end-to-end against the reference at the 2e-2 gate)."""
    eng = nc.scalar
    ins = [eng.lower_ap(in_)]
    for val in (0.0, 1.0, 0.0):  # bias, scale, alpha
        ins.append(mybir.ImmediateValue(dtype=mybir.dt.float32, value=val))
    return eng.add_instruction(
        mybir.InstActivation(
            name=eng.bass.get_next_instruction_name(),
            func=AF.Reciprocal,
            ins=ins,
            outs=[eng.lower_ap(out)],
        )
    )


def _dve_recip_bf16(nc, out, in_):
    """reciprocal_approx_fast with bf16 APs (the bass fp32 assert is
    over-strict: DVE converts streams to fp32 before the uop chain, so the
    BITWISE_NOT seed still sees fp32 bit layout)."""
    from concourse.dve_ops import (
        RECIP_APPROX_FAST_CONSTS,
        RECIPROCAL_APPROX_FAST,
    )
    c = RECIP_APPROX_FAST_CONSTS
    return nc.vector._custom_dve(
        RECIPROCAL_APPROX_FAST, out=out, in0=in_,
        s0=c["s0"], s1=c["s1"], imm2=c["imm2"])


# ---------------------------------------------------------------- builder
def build_nc(sc):
    cust = _register_custom_ops()
    assert len(cust) == 3, "custom DVE ops required for this kernel"
    nc = bacc.Bacc(None)

    xT = nc.dram_tensor("xT", [D_IN, BC], bf16, kind="ExternalInput")
    W1d = nc.dram_tensor("W1e", [D_IN, HP], bf16, kind="ExternalInput")
    W2d = nc.dram_tensor("W2e", [HP, HP], bf16, kind="ExternalInput")
    W3d = nc.dram_tensor("W3e", [HP, K], bf16, kind="ExternalInput")
    mu0d = nc.dram_tensor("mu0", [P, T], bf16, kind="ExternalInput")
    s20d = nc.dram_tensor("s20", [P, T], bf16, kind="ExternalInput")
    # k-major bf16 output: out[g, p, k*TG+t] = result[(g*TG+t)*P+p, k];
    # the host transposes/upcasts (keeps every DMA fully contiguous).
    outd = nc.dram_tensor("out", [G, P, K * TG], bf16, kind="ExternalOutput")

    A_ = sc["ns"] * sc["a_mu"] * (1.0 + 1.0 / sc["nu"])
    C_ = sc["ns"] * sc["a_s"] * (1.0 + 1.0 / sc["nu"])
    D_ = sc["b_s"] - sc["ns"] * sc["a_s"]
    Ct = sc["nu"] * C_
    wt = sc["nu"] * sc["o_s"]

    KB = K
    XR = D_IN - P  # 72 rows in the second x slab
    with tile.TileContext(nc) as tc:
        with (
            tc.tile_pool(name="const", bufs=1) as constp,
            tc.tile_pool(name="big", bufs=1) as bigp,
            tc.tile_pool(name="mlp", bufs=XBUFS) as mlpp,
            tc.tile_pool(name="act", bufs=2) as actp,
            tc.tile_pool(name="scan", bufs=3) as scanp,
            tc.tile_pool(name="psmm", bufs=3, space="PSUM") as psmm,
            tc.tile_pool(name="psdp", bufs=2, space="PSUM") as psdp,
        ):
            # ---- constants
            w1a = constp.tile([P, HP], bf16, tag="w1a")
            nc.sync.dma_start(w1a[:], W1d[0:P, :])
            w1b = constp.tile([XR, HP], bf16, tag="w1b")
            nc.sync.dma_start(w1b[:], W1d[P:D_IN, :])
            w2 = constp.tile([HP, HP], bf16, tag="w2")
            nc.sync.dma_start(w2[:], W2d[:])
            w3 = constp.tile([HP, K], bf16, tag="w3")
            nc.sync.dma_start(w3[:], W3d[:])
            zt = constp.tile([P, 1], f32, tag="zt")
            nc.vector.memset(zt[:], 0.0)

            # ---- persistent combined state tiles, k-major within group:
            # DP col g*KB*TG + k*TG + t; MU/QQ col g*(KB+1)*TG + k*TG + t.
            # One tile per state so a single step-sliced AP can address both
            # groups at a constant slab offset (the skew-fused scan).
            DP = bigp.tile([P, G * KB * TG], bf16, tag="DP", name="DP")
            MU = bigp.tile([P, G * (KB + 1) * TG], bf16, tag="MU", name="MU")
            QQ = bigp.tile([P, G * (KB + 1) * TG], bf16, tag="QQ", name="QQ")

            def dp_slab(g, k, n=1):
                b = g * KB * TG
                return DP[:, b + k * TG:b + (k + n) * TG]

            def mu_slab(g, k, n=1):
                b = g * (KB + 1) * TG
                return MU[:, b + k * TG:b + (k + n) * TG]

            def q_slab(g, k, n=1):
                b = g * (KB + 1) * TG
                return QQ[:, b + k * TG:b + (k + n) * TG]

            def mlp_chunk(g, c, evac_l2_dve=False):
                c_glob = g * CPG + c
                col0 = c_glob * CHUNK
                xa = mlpp.tile([P, CHUNK], bf16, tag="xa")
                nc.sync.dma_start(xa[:], xT[0:P, col0:col0 + CHUNK])
                xb = mlpp.tile([XR, CHUNK], bf16, tag="xb")
                nc.sync.dma_start(xb[:], xT[P:D_IN, col0:col0 + CHUNK])

                r1 = actp.tile([HP, CHUNK], bf16, tag="r1")
                r2 = actp.tile([HP, CHUNK], bf16, tag="r2")

                ps1 = psmm.tile([HP, CHUNK], f32, tag="mm")
                for j in range(CHUNK // MM_N):
                    s = slice(j * MM_N, (j + 1) * MM_N)
                    nc.tensor.matmul(ps1[:, s], w1a[:], xa[:, s],
                                     start=True, stop=False)
                for j in range(CHUNK // MM_N):
                    s = slice(j * MM_N, (j + 1) * MM_N)
                    nc.tensor.matmul(ps1[:, s], w1b[:], xb[:, s],
                                     start=False, stop=True)
                nc.scalar.activation(r1[:], ps1[:], AF.Relu,
                                     bias=zt[:, 0:1])

                ps2 = psmm.tile([HP, CHUNK], f32, tag="mm")
                for j in range(CHUNK // MM_N):
                    s = slice(j * MM_N, (j + 1) * MM_N)
                    nc.tensor.matmul(ps2[:, s], w2[:], r1[:, s],
                                     start=True, stop=True)
                if evac_l2_dve:
                    nc.vector.tensor_scalar_max(r2[:], ps2[:], 0.0)
                else:
                    nc.scalar.activation(r2[:], ps2[:], AF.Relu,
                                         bias=zt[:, 0:1])

                psd = psdp.tile([P, (CHUNK // P) * KB], f32, tag="dp")
                for j in range(CHUNK // P):
                    nc.tensor.matmul(psd[:, j * KB:(j + 1) * KB],
                                     r2[:, j * P:(j + 1) * P], w3[:],
                                     start=True, stop=True)
                # evac to k-major dp: dp[:, k*TG + c*8+j] = psd[:, j*KB+k]
                src = psd[:].rearrange("p (j k) -> p k j", k=KB)
                dst = dp_slab(g, 0, KB).rearrange("p (k t) -> p k t", t=TG)
                dst = dst[:, :, c * (CHUNK // P):(c + 1) * (CHUNK // P)]
                nc.scalar.copy(dst, src)

            def scan_ops(yv, mu_p, mu_n, Q_p, Q_n, width):
                """One GAS step on the given APs (solo or fused width)."""
                def tt(name, out, a, b, op):
                    eng = nc.gpsimd if name in POOL_OPS else nc.vector
                    eng.tensor_tensor(out, a, b, op)

                def scr(tag):
                    t = scanp.tile([P, width * TG], bf16, tag=f"{tag}{width}")
                    flat = t[:]
                    if width == 1:
                        return flat, flat
                    return flat.rearrange("p (x t) -> p x t", t=TG), flat

                e, _ = scr("e")
                tt("e", e, yv, mu_p, ALU.subtract)
                d, _ = scr("d")
                nc.vector._custom_dve(cust["ARGAS_SQ_ADD"],
                                      out=d, in0=e, in1=Q_p)
                p, _ = scr("p")
                tt("p", p, e, Q_p, ALU.mult)
                r, r_flat = scr("r")
                _dve_recip_bf16(nc, r, d)
                m1, m1_flat = scr("m1")
                tt("m", m1, p, r, ALU.mult)
                nc.vector._custom_dve(cust["ARGAS_AFF_AFF2"],
                                      out=mu_n, in0=mu_p, in1=m1_flat,
                                      s0=A_, s1=sc["o_mu"], imm2=sc["b_mu"])
                nc.vector._custom_dve(cust["ARGAS_QP"],
                                      out=Q_n, in0=Q_p, in1=r_flat,
                                      s0=-Ct, s1=Ct + D_, imm2=wt)

            def scan_solo(g, k):
                scan_ops(dp_slab(g, k), mu_slab(g, k), mu_slab(g, k + 1),
                         q_slab(g, k), q_slab(g, k + 1), 1)

            def scan_fused(k):
                """g0 at step k, g1 at step k-SKEW, one AP pair per operand."""
                XD, XM = KB - SKEW, KB + 1 - SKEW
                dpv = DP[:].rearrange("p (x t) -> p x t", t=TG)
                muv = MU[:].rearrange("p (x t) -> p x t", t=TG)
                qv = QQ[:].rearrange("p (x t) -> p x t", t=TG)
                scan_ops(
                    dpv[:, k:k + XD + 1:XD, :],
                    muv[:, k:k + XM + 1:XM, :],
                    muv[:, k + 1:k + 1 + XM + 1:XM, :],
                    qv[:, k:k + XM + 1:XM, :],
                    qv[:, k + 1:k + 1 + XM + 1:XM, :], 2)

            def scan_init(g):
                nc.sync.dma_start(mu_slab(g, 0),
                                  mu0d[:, g * TG:(g + 1) * TG])
                nc.sync.dma_start(q_slab(g, 0),
                                  s20d[:, g * TG:(g + 1) * TG])

            def tail_slab(g, k, add_pool=True):
                """sg=sqrt(Q/nu) on ACT, dp = dp*sg + mu in place (slab k)."""
                sgk = q_slab(g, k + 1)
                nc.scalar.activation(sgk, sgk, AF.Sqrt,
                                     bias=zt[:, 0:1], scale=1.0 / sc["nu"])
                dk = dp_slab(g, k)
                nc.gpsimd.tensor_tensor(dk, dk, sgk, ALU.mult)
                eng = nc.gpsimd if add_pool else nc.vector
                eng.tensor_tensor(dk, dk, mu_slab(g, k + 1), ALU.add)

            def tail_bulk(g, k0, k1):
                sgk = q_slab(g, k0 + 1, k1 - k0)
                nc.scalar.activation(sgk, sgk, AF.Sqrt,
                                     bias=zt[:, 0:1], scale=1.0 / sc["nu"])
                dk = dp_slab(g, k0, k1 - k0)
                nc.vector.tensor_tensor(dk, dk, sgk, ALU.mult)
                nc.vector.tensor_tensor(dk, dk, mu_slab(g, k0 + 1, k1 - k0),
                                        ALU.add)

            def dma_out(g, k0, k1):
                nc.sync.dma_start(outd[g, :, k0 * TG:k1 * TG],
                                  dp_slab(g, k0, k1 - k0))

            # ---------------- emission schedule ----------------
            for g in range(G):
                scan_init(g)
            for c in range(CPG):          # group 0 MLP; L2 relu on idle DVE
                mlp_chunk(0, c, evac_l2_dve=True)

            # phase B: solo g0 steps 0..SKEW-1, mlp-g1 interleaved
            nxt_c = 0
            per = max(1, SKEW // CPG)
            for k in range(SKEW):
                scan_solo(0, k)
                if (k + 1) % per == 0 and nxt_c < CPG:
                    mlp_chunk(1, nxt_c)
                    nxt_c += 1
            while nxt_c < CPG:
                mlp_chunk(1, nxt_c)
                nxt_c += 1
            # phase C: fused (g0 at k, g1 at k-SKEW); g0 tail slabs on
            # Pool/ACT as they become final
            for k in range(SKEW, KB):
                scan_fused(k)
                tail_slab(0, k - SKEW)
            dma_out(0, 0, KB - SKEW)
            # phase D: solo g1 drain + rest of g0 tail (keeping g1's tail
            # out of D: Pool/DVE SBUF contention measurably slows the scan)
            for k in range(KB - SKEW, KB):
                scan_solo(1, k)
                tail_slab(0, k)
            dma_out(0, KB - SKEW, KB)
            # phase E: g1 tail in two pipelined k-slabs
            for h in range(2):
                k0, k1 = h * KB // 2, (h + 1) * KB // 2
                tail_bulk(1, k0, k1)
                dma_out(1, k0, k1)
    if not nc.is_finalized():
        nc.finalize()
    return nc


# ---------------------------------------------------------------- tracing
def _maybe_enable_trace():
    if os.environ.get("BASS_TRACE") != "1":
        return
    try:
        import sys, types
        try:
            import antenv.axon_hooks as ah
        except ImportError:
            import antenv
            ah = types.ModuleType("antenv.axon_hooks")
            ah._hook = None
            def _set(h):
                ah._hook = h
            def _get():
                return ah._hook
            ah.set_axon_ntff_profile_hook = _set
            ah.get_axon_ntff_profile_hook = _get
            sys.modules["antenv.axon_hooks"] = ah
            antenv.axon_hooks = ah
        if ah.get_axon_ntff_profile_hook() is not None:
            return
        from trn_agent_boot.trn_boot import _ntff_profile_via_ctypes
        import concourse.bass_utils as bu
        bu.upload_artifacts = lambda tmpdir: tmpdir
        ah.set_axon_ntff_profile_hook(
            _ntff_profile_via_ctypes("/opt/axon/libaxon_pjrt.so"))
        print("[kernel] NTFF profile hook installed")
    except Exception as e:
        print(f"[kernel] trace hook unavailable: {e}")


LAST = None  # last BassKernelResults (dev/tracing)


# ---------------------------------------------------------------- entry
def kernel(**inputs):
    import ml_dtypes
    bfl = ml_dtypes.bfloat16
    _maybe_enable_trace()
    x = np.asarray(inputs["x"], np.float32)
    last_mu = np.asarray(inputs["last_mu"], np.float32)
    last_sigma = np.asarray(inputs["last_sigma"], np.float32)
    sc = dict(
        a_mu=float(inputs["alpha_mu"]), a_s=float(inputs["alpha_sigma"]),
        b_mu=float(inputs["beta_mu"]), b_s=float(inputs["beta_sigma"]),
        o_mu=float(inputs["omega_mu"]), o_s=float(inputs["omega_sigma"]),
        nu=float(inputs["nu"]), ns=float(inputs["norm_strength"]),
    )
    # biases are structurally zero in this net (setup_inputs); the padded
    # no-bias-row layout depends on it.
    for bn in ("b1", "b2", "b3"):
        assert float(np.abs(np.asarray(inputs[bn])).max()) == 0.0, \
            f"{bn} != 0 unsupported by padded kernel"

    def pad(w, rows, cols):
        out = np.zeros((rows, cols), np.float32)
        a = np.asarray(w, np.float32)
        out[:a.shape[0], :a.shape[1]] = a
        return out.astype(bfl)

    W1e = pad(inputs["W1"], D_IN, HP)
    W2e = pad(inputs["W2"], HP, HP)
    W3e = pad(inputs["W3"], HP, K)

    nc = build_nc(sc)
    in_maps = []
    for c in range(NCORES):
        sl = slice(c * BC, (c + 1) * BC)
        in_maps.append({
            "xT": np.ascontiguousarray(x[sl].T).astype(bfl),
            "W1e": W1e, "W2e": W2e, "W3e": W3e,
            "mu0": np.ascontiguousarray(
                last_mu[sl].reshape(T, P).T).astype(bfl),
            "s20": np.ascontiguousarray(
                sc["nu"] * last_sigma[sl].reshape(T, P).T).astype(bfl),
        })
    res = run_bass_kernel_spmd(nc, in_maps, list(range(NCORES)))
    global LAST
    LAST = res
    if res.exec_time_ns is not None:
        print(f"HW exec time: {res.exec_time_ns} ns")
    # out[g, p, k*TG+t] -> full[(g*TG+t)*P+p, k]
    parts = []
    for i in range(NCORES):
        o = np.asarray(res.results[i]["out"]).astype(np.float32)
        o = o.reshape(G, P, K, TG).transpose(0, 3, 1, 2).reshape(BC, K)
        parts.append(o)
    return np.concatenate(parts, 0)
